# revision 6
# baseline (speedup 1.0000x reference)
"""Trainium2 Bass kernel for DualThresholdSelfregulatingIntegrate.

Computes, matching the jax-on-neuron reference to ~1 moved spike:
    rates  = relu(x) * DT                     # [B, T, D]
    c      = init[:, None, :] + cumsum(rates, axis=1)
    spikes = floor(c) - floor(c_prev)
    out    = spikes / DT

Architecture (per core, pure data-parallel over batch):
  - x loads in natural [t, d] layout; ScalarE computes rates=relu(x)*dt
  - per 128-step chunk, one fp32 PE matmul per d-block computes the
    chunk-local cumsum AND the [t,d]->[d,t'] transpose in one op:
    L[d, t'] = sum_t rates[t, d] * U[t, t'] with U upper-tri ones;
    fp32 PE accumulation reproduces the reference lowering's rounding
  - carries across chunks are Kahan-compensated sums of the chunk totals
    (exact), held per [d_inner(128), j(8)]; the chain starts at v0 - 0.5
    so F = rint(L + carry) = floor(c) directly (int16); per chunk this is
    one DVE STT (carry broadcast along t') and/or per-j ScalarE
    activations (carry as per-partition bias)
  - spike diff along the free (t) axis in {0,1} (int16 -> fp16), written
    per 4-chunk group and DMA'd out in [d, t] layout; the host applies
    the pure layout transpose back to [t, d] and scales {0,1}->{0,1000}
  - output is fp16 (exact for {0,1000}), halving output HBM traffic;
    input DMA alternates between the SP and Pool queues, output DMA
    likewise, so no engine queue carries more than ~38us of DMA
"""

import sys

sys.path.insert(0, "/opt/trn_rl_repo")

import numpy as np

import concourse.bass as bass  # noqa: F401  (registers engines)
import concourse.tile as tile
from concourse import bacc, mybir

N_CORES = 8
B, T, D = 16, 2048, 1024
BC = B // N_CORES          # batches per core
CH = 128                   # chunk (carry granularity, matmul contraction)
G = 512                    # group: time steps per pipeline stage
CPG = G // CH              # 4 chunks per group
NG = T // G                # 4 groups per batch
NDB = D // CH              # 8 d-blocks
dt = mybir.dt

_cache = {}
CFG = {
    "na": 3,        # first `na` d-blocks' F on ScalarE (bias trick), rest DVE
    "relun": 2,     # chunks per relu op
    "kaheng": "v",
    "pcb": 4,       # pc PSUM bufs (2 banks each)
    "ldq": "sg",    # per-group load queue cycle: s=sync, g=gpsimd, a=scalar
    "stq": "gs",    # per-group store queue cycle
}
ENGQ = {"v": "vector", "g": "gpsimd", "s": "sync", "a": "scalar"}


def build_nc(bench_iters=0, skip=()):
    nc = bacc.Bacc("TRN2", target_bir_lowering=False, debug=False)
    x = nc.dram_tensor("x", [BC, T, D], dt.float32, kind="ExternalInput")
    v0t = nc.dram_tensor("v0t", [BC, CH, NDB], dt.float32, kind="ExternalInput")
    u = nc.dram_tensor("u", [CH, CH], dt.float32, kind="ExternalInput")
    y = nc.dram_tensor("y", [BC, D, T], dt.float16, kind="ExternalOutput")

    with tile.TileContext(nc) as tc:
        with tc.tile_pool(name="xin", bufs=2) as xin_p, \
             tc.tile_pool(name="rt", bufs=2) as rt_p, \
             tc.tile_pool(name="ff", bufs=2) as f_p, \
             tc.tile_pool(name="sp", bufs=2) as sp_p, \
             tc.tile_pool(name="cr", bufs=2) as cr_p, \
             tc.tile_pool(name="sm", bufs=3) as sm_p, \
             tc.tile_pool(name="consts", bufs=1) as c_p, \
             tc.tile_pool(name="pc", bufs=CFG["pcb"], space="PSUM") as pc_p:

            ut = c_p.tile([CH, CH], dt.float32, tag="ut")
            nc.sync.dma_start(ut[:], u[:])
            v0tt = c_p.tile([CH, BC * NDB], dt.float32, tag="v0tt")
            nc.sync.dma_start(
                v0tt[:].rearrange("p (b j) -> p b j", b=BC),
                v0t[:].rearrange("b p j -> p b j"),
            )
            v03 = v0tt[:].rearrange("p (b j) -> p b j", b=BC)

            import contextlib
            _hints = (mybir.EngineType.DVE, mybir.EngineType.Activation,
                      mybir.EngineType.PE, mybir.EngineType.SP,
                      mybir.EngineType.Pool)
            loop_cm = tc.For_i(0, bench_iters, 1, hint_engines=_hints) \
                if bench_iters else contextlib.nullcontext()
            with loop_cm:
                body(nc, tc, x, y, v03, ut,
                     xin_p, rt_p, f_p, sp_p, cr_p, sm_p, pc_p,
                     skip=set(skip))
    nc.compile()
    return nc


def body(nc, tc, x, y, v03, ut,
         xin_p, rt_p, f_p, sp_p, cr_p, sm_p, pc_p, skip=()):
    AL = mybir.AluOpType
    AF = mybir.ActivationFunctionType
    kah = getattr(nc, ENGQ[CFG["kaheng"]])
    NA = CFG["na"]
    RN = CFG["relun"]

    for b in range(BC):
        comp_old = None
        f_prev = None
        carr_prev = None
        for g in range(NG):
            t0 = g * G
            # ---- load x group [128, 4, 1024] (natural layout) ----
            _le = getattr(nc, ENGQ[CFG["ldq"][g % len(CFG["ldq"])]])
            x4 = xin_p.tile([CH, CPG * D], dt.float32, tag="xk")
            _le.dma_start(
                x4[:].rearrange("p (c d) -> p c d", c=CPG),
                x[b, t0:t0 + G, :].rearrange("(c p) d -> p c d", p=CH))

            # ---- rates (natural layout), relu*dt on ScalarE ----
            r4 = rt_p.tile([CH, CPG * D], dt.float32, tag="rk")
            for c in range(0, CPG, RN):
                nc.scalar.activation(r4[:, c * D:(c + RN) * D],
                                     x4[:, c * D:(c + RN) * D],
                                     AF.Relu, bias=0.0, scale=0.001)

            # ---- carry tile: c3[:, c, :] = v0-0.5 + Kahan sum of totals
            #      of all chunks before (g, c); col CPG feeds next group ----
            carr = cr_p.tile([CH, (CPG + 1) * NDB], dt.float32, tag="carr")
            c3 = carr[:].rearrange("p (c j) -> p c j", c=CPG + 1)
            if g == 0:
                nc.vector.tensor_scalar(c3[:, 0, :], v03[:, b, :], -0.5, None,
                                        op0=AL.add)
                comp_old = sm_p.tile([CH, NDB], dt.float32, tag="ckah")
                nc.vector.memset(comp_old[:], 0.0)
            else:
                nc.vector.tensor_copy(c3[:, 0, :], carr_prev[:, CPG, :])

            # ---- F tile for the group (col 0 = prev chunk boundary) ----
            ft = f_p.tile([CH, NDB * (G + 1)], dt.int16, tag="fk")
            f3 = ft[:].rearrange("p (j t) -> p j t", j=NDB)
            if g == 0:
                nc.vector.memset(f3[:, :, 0], 0)
            else:
                nc.vector.tensor_copy(f3[:, :, 0], f_prev[:, :, G])

            for c in range(CPG):
                # chunk-local cumsum + transpose via one matmul per d-block
                pck = pc_p.tile([CH, D], dt.float32, tag="pck")
                for j in range(NDB):
                    nc.tensor.matmul(pck[:, j * CH:(j + 1) * CH],
                                     r4[:, c * D + j * CH:c * D + (j + 1) * CH],
                                     ut[:], start=True, stop=True)
                pc3 = pck[:].rearrange("p (j t) -> p j t", j=NDB)

                # F = rint(L + carr) = floor(c), int16
                if "f" not in skip:
                    for j in range(NA):
                        nc.scalar.activation(
                            f3[:, j, 1 + c * CH:1 + (c + 1) * CH],
                            pc3[:, j, :], AF.Relu,
                            bias=c3[:, c, j:j + 1], scale=1.0)
                    if NA < NDB:
                        cb = c3[:, c, NA:].unsqueeze(2).broadcast_to(
                            [CH, NDB - NA, CH])
                        nc.vector.scalar_tensor_tensor(
                            f3[:, NA:, 1 + c * CH:1 + (c + 1) * CH],
                            pc3[:, NA:, :], 1.0, cb,
                            op0=AL.mult, op1=AL.add)

                # Kahan-accumulate chunk totals into the carry chain
                tot = pc3[:, :, CH - 1]
                ykh = sm_p.tile([CH, NDB], dt.float32, tag="ykah")
                kah.tensor_tensor(ykh[:], tot, comp_old[:], op=AL.subtract)
                kah.tensor_tensor(c3[:, c + 1, :], c3[:, c, :], ykh[:],
                                  op=AL.add)
                dkh = sm_p.tile([CH, NDB], dt.float32, tag="dkah")
                kah.tensor_tensor(dkh[:], c3[:, c + 1, :], c3[:, c, :],
                                  op=AL.subtract)
                comp_new = sm_p.tile([CH, NDB], dt.float32, tag="ckah")
                kah.tensor_tensor(comp_new[:], dkh[:], ykh[:],
                                  op=AL.subtract)
                comp_old = comp_new

            carr_prev = c3
            f_prev = f3

            # ---- spikes = F_t - F_{t-1} in {0,1}, fp16, [d, t] layout ----
            spk = sp_p.tile([CH, NDB * G], dt.float16, tag="sk")
            s3 = spk[:].rearrange("p (j t) -> p j t", j=NDB)
            nc.vector.tensor_tensor(s3, f3[:, :, 1:G + 1], f3[:, :, 0:G],
                                    op=AL.subtract)

            # ---- store group in [d, t] layout (host transposes) ----
            if "out" in skip:
                continue
            _se = getattr(nc, ENGQ[CFG["stq"][g % len(CFG["stq"])]])
            _se.dma_start(
                y[b, :, t0:t0 + G].rearrange("(j p) t -> p j t", p=CH),
                s3)


def _get_nc():
    if "nc" not in _cache:
        _cache["nc"] = build_nc()
    return _cache["nc"]


def _make_in_maps(x, v0):
    uv = np.triu(np.ones((CH, CH), dtype=np.float32))
    in_maps = []
    for c in range(N_CORES):
        xb = np.ascontiguousarray(x[BC * c:BC * (c + 1)])
        v0b = v0[BC * c:BC * (c + 1)]
        v0tb = np.ascontiguousarray(
            v0b.reshape(BC, NDB, CH).transpose(0, 2, 1).astype(np.float32))
        in_maps.append({"x": xb, "v0t": v0tb, "u": uv})
    return in_maps


def _get_runner():
    """Build (once) a cached jitted SPMD executable over the 8 cores."""
    if "runner" in _cache:
        return _cache["runner"]
    import jax
    from jax.sharding import Mesh, PartitionSpec
    from jax.experimental.shard_map import shard_map
    from concourse import bass2jax

    nc = _get_nc()
    bass2jax.install_neuronx_cc_hook()
    partition_name = nc.partition_id_tensor.name if nc.partition_id_tensor else None
    in_names, out_names, out_avals = [], [], []
    for alloc in nc.m.functions[0].allocations:
        if not isinstance(alloc, mybir.MemoryLocationSet):
            continue
        name = alloc.memorylocations[0].name
        if alloc.kind == "ExternalInput":
            if name != partition_name:
                in_names.append(name)
        elif alloc.kind == "ExternalOutput":
            out_names.append(name)
            out_avals.append(jax.core.ShapedArray(
                tuple(alloc.tensor_shape), dt.np(alloc.dtype)))
    n_params = len(in_names)
    zero_outs = [np.zeros(a.shape, a.dtype) for a in out_avals]
    all_names = in_names + out_names + ([partition_name] if partition_name else [])

    def _body(*args):
        operands = list(args)
        if partition_name is not None:
            operands.append(bass2jax.partition_id_tensor())
        return tuple(bass2jax._bass_exec_p.bind(
            *operands, out_avals=tuple(out_avals), in_names=tuple(all_names),
            out_names=tuple(out_names), lowering_input_output_aliases=(),
            sim_require_finite=True, sim_require_nnan=True, nc=nc))

    devices = jax.devices()[:N_CORES]
    mesh = Mesh(np.asarray(devices), ("core",))
    nin = n_params + len(out_names)
    fn = jax.jit(shard_map(_body, mesh=mesh,
                           in_specs=(PartitionSpec("core"),) * nin,
                           out_specs=(PartitionSpec("core"),) * len(out_names),
                           check_rep=False))
    _cache["runner"] = (fn, in_names, out_names, zero_outs)
    return _cache["runner"]


def kernel(inputs, initial_state):
    import jax
    x = np.ascontiguousarray(np.asarray(inputs, dtype=np.float32))
    v0 = np.ascontiguousarray(np.asarray(initial_state, dtype=np.float32))
    assert x.shape == (B, T, D) and v0.shape == (B, D)
    fn, in_names, out_names, zero_outs = _get_runner()
    in_maps = _make_in_maps(x, v0)
    concat_in = [np.concatenate([np.asarray(in_maps[c][nm])
                                 for c in range(N_CORES)], axis=0)
                 for nm in in_names]
    concat_zero = [np.concatenate([z] * N_CORES, axis=0) for z in zero_outs]
    outs = jax.block_until_ready(fn(*concat_in, *concat_zero))
    ydt = np.asarray(outs[out_names.index("y")])        # [B, D, T] fp16 {0,1}
    out = ydt.astype(np.float32).transpose(0, 2, 1) * np.float32(1000.0)
    return np.ascontiguousarray(out)


# revision 12
# speedup vs baseline: 1.2102x; 1.2102x over previous
"""Trainium2 Bass kernel for DualThresholdSelfregulatingIntegrate.

Computes, matching the jax-on-neuron reference to ~1 moved spike:
    rates  = relu(x) * DT                     # [B, T, D]
    c      = init[:, None, :] + cumsum(rates, axis=1)
    spikes = floor(c) - floor(c_prev)
    out    = spikes / DT

Architecture (per core, pure data-parallel over batch):
  - x loads in natural [t, d] layout; ScalarE computes rates=relu(x)*dt
  - per 128-step chunk, one fp32 PE matmul per d-block computes the
    chunk-local cumsum AND the [t,d]->[d,t'] transpose in one op:
    L[d, t'] = sum_t rates[t, d] * U[t, t'] with U upper-tri ones;
    fp32 PE accumulation reproduces the reference lowering's rounding
  - carries across chunks are Kahan-compensated sums of the chunk totals
    (exact), held per [d_inner(128), j(8)]; the chain starts at v0 - 0.5
    so F = rint(L + carry) = floor(c) directly (int16); per chunk this is
    one DVE STT (carry broadcast along t') and/or per-j ScalarE
    activations (carry as per-partition bias)
  - spike diff along the free (t) axis in {0,1} (int16 -> fp16), written
    per 4-chunk group and DMA'd out in [d, t] layout; the host applies
    the pure layout transpose back to [t, d] and scales {0,1}->{0,1000}
  - output is fp16 (exact for {0,1000}), halving output HBM traffic;
    input DMA alternates between the SP and Pool queues, output DMA
    likewise, so no engine queue carries more than ~38us of DMA
"""

import sys

sys.path.insert(0, "/opt/trn_rl_repo")

import numpy as np

import concourse.bass as bass  # noqa: F401  (registers engines)
import concourse.tile as tile
from concourse import bacc, mybir

N_CORES = 8
B, T, D = 16, 2048, 1024
BC = B // N_CORES          # batches per core
CH = 128                   # chunk (carry granularity, matmul contraction)
G = 512                    # group: time steps per pipeline stage
CPG = G // CH              # 4 chunks per group
NG = T // G                # 4 groups per batch
NDB = D // CH              # 8 d-blocks
dt = mybir.dt

_cache = {}
CFG = {
    "na": 0,        # first `na` d-blocks' F on ScalarE (bias trick), rest DVE
    "relun": 1,     # chunks per relu op
    "kaheng": "g",
    "pcb": 4,       # pc PSUM bufs (2 banks each)
    "ldq": "sg",    # per-group load queue cycle: s=sync, g=gpsimd, a=scalar
    "stq": "gs",    # per-group store queue cycle
}
ENGQ = {"v": "vector", "g": "gpsimd", "s": "sync", "a": "scalar"}


def build_nc(bench_iters=0, skip=()):
    nc = bacc.Bacc("TRN2", target_bir_lowering=False, debug=False)
    x = nc.dram_tensor("x", [BC, T, D], dt.float32, kind="ExternalInput")
    v0t = nc.dram_tensor("v0t", [BC, CH, NDB], dt.float32, kind="ExternalInput")
    u = nc.dram_tensor("u", [CH, CH], dt.float32, kind="ExternalInput")
    y = nc.dram_tensor("y", [BC, D, T], dt.int16, kind="ExternalOutput")

    with tile.TileContext(nc) as tc:
        with tc.tile_pool(name="xin", bufs=3) as xin_p, \
             tc.tile_pool(name="rt", bufs=3) as rt_p, \
             tc.tile_pool(name="ff", bufs=3) as f_p, \
             tc.tile_pool(name="sp", bufs=2) as sp_p, \
             tc.tile_pool(name="cr", bufs=3) as cr_p, \
             tc.tile_pool(name="sm", bufs=5) as sm_p, \
             tc.tile_pool(name="consts", bufs=1) as c_p, \
             tc.tile_pool(name="pc", bufs=CFG["pcb"], space="PSUM") as pc_p:

            ut = c_p.tile([CH, CH], dt.float32, tag="ut")
            nc.sync.dma_start(ut[:], u[:])
            v0tt = c_p.tile([CH, BC * NDB], dt.float32, tag="v0tt")
            nc.sync.dma_start(
                v0tt[:].rearrange("p (b j) -> p b j", b=BC),
                v0t[:].rearrange("b p j -> p b j"),
            )
            v03 = v0tt[:].rearrange("p (b j) -> p b j", b=BC)

            import contextlib
            _hints = (mybir.EngineType.DVE, mybir.EngineType.Activation,
                      mybir.EngineType.PE, mybir.EngineType.SP,
                      mybir.EngineType.Pool)
            loop_cm = tc.For_i(0, bench_iters, 1, hint_engines=_hints) \
                if bench_iters else contextlib.nullcontext()
            with loop_cm:
                body(nc, tc, x, y, v03, ut,
                     xin_p, rt_p, f_p, sp_p, cr_p, sm_p, pc_p,
                     skip=set(skip))
    nc.compile()
    return nc


def body(nc, tc, x, y, v03, ut,
         xin_p, rt_p, f_p, sp_p, cr_p, sm_p, pc_p, skip=()):
    """Software-pipelined emission: for pipeline step i, the produce stage
    (DMA + relu) of group i is emitted BEFORE the consume stage (matmuls,
    floor, carry, diff, store) of group i-1, so ScalarE's relu of the next
    group is never queued behind floor ops that wait on PE."""
    AL = mybir.AluOpType
    AF = mybir.ActivationFunctionType
    kah = getattr(nc, ENGQ[CFG["kaheng"]])
    NA = CFG["na"]
    RN = CFG["relun"]
    NGRP = BC * NG
    state = {}
    bstate = {0: {}, 1: {}}

    def produce(i):
        g, b = divmod(i, BC)
        t0 = g * G
        x4 = xin_p.tile([CH, CPG * D], dt.float32, tag="xk")
        r4 = rt_p.tile([CH, CPG * D], dt.float32, tag="rk")
        for c in range(0, CPG, RN):
            _le = getattr(nc, ENGQ[CFG["ldq"][(2 * i + c // RN) % len(CFG["ldq"])]])
            _le.dma_start(
                x4[:, c * D:(c + RN) * D].rearrange("p (c d) -> p c d", c=RN),
                x[b, t0 + c * CH:t0 + (c + RN) * CH, :]
                .rearrange("(c p) d -> p c d", p=CH))
            nc.scalar.activation(r4[:, c * D:(c + RN) * D],
                                 x4[:, c * D:(c + RN) * D],
                                 AF.Relu, bias=0.0, scale=0.001)
        state[i] = r4

    def consume(i):
        g, b = divmod(i, BC)
        t0 = g * G
        r4 = state.pop(i)
        bst = bstate[b]

        # carry tile: c3[:, c, :] = v0-0.5 + Kahan sum of totals of all
        # chunks before (g, c); col CPG feeds the next group
        carr = cr_p.tile([CH, (CPG + 1) * NDB], dt.float32, tag="carr")
        c3 = carr[:].rearrange("p (c j) -> p c j", c=CPG + 1)
        with tc.high_priority():
            if g == 0:
                nc.vector.tensor_scalar(c3[:, 0, :], v03[:, b, :], -0.5, None,
                                        op0=AL.add)
                comp_old = sm_p.tile([CH, NDB], dt.float32, tag="ckah")
                nc.vector.memset(comp_old[:], 0.0)
            else:
                nc.vector.tensor_copy(c3[:, 0, :], bst["carr"][:, CPG, :])
                comp_old = bst["comp"]

        # F tile for the group (col 0 = prev chunk boundary floor)
        ft = f_p.tile([CH, NDB * (G + 1)], dt.int16, tag="fk")
        f3 = ft[:].rearrange("p (j t) -> p j t", j=NDB)
        with tc.high_priority():
            if g == 0:
                nc.vector.memset(f3[:, :, 0], 0)
            else:
                nc.vector.tensor_copy(f3[:, :, 0], bst["f"][:, :, G])

        spk = sp_p.tile([CH, NDB * G], dt.int16, tag="sk")
        s3 = spk[:].rearrange("p (j t) -> p j t", j=NDB)

        for c in range(CPG):
            # chunk-local cumsum + transpose via one matmul per d-block
            pck = pc_p.tile([CH, D], dt.float32, tag="pck")
            for j in range(NDB):
                nc.tensor.matmul(pck[:, j * CH:(j + 1) * CH],
                                 r4[:, c * D + j * CH:c * D + (j + 1) * CH],
                                 ut[:], start=True, stop=True)
            pc3 = pck[:].rearrange("p (j t) -> p j t", j=NDB)

            with tc.high_priority():
                # F = rint(L + carr) = floor(c), int16
                if "f" not in skip:
                    for j in range(NA):
                        nc.scalar.activation(
                            f3[:, j, 1 + c * CH:1 + (c + 1) * CH],
                            pc3[:, j, :], AF.Relu,
                            bias=c3[:, c, j:j + 1], scale=1.0)
                    if NA < NDB:
                        cb = c3[:, c, NA:].unsqueeze(2).broadcast_to(
                            [CH, NDB - NA, CH])
                        nc.vector.scalar_tensor_tensor(
                            f3[:, NA:, 1 + c * CH:1 + (c + 1) * CH],
                            pc3[:, NA:, :], 1.0, cb,
                            op0=AL.mult, op1=AL.add)

                # Kahan-accumulate chunk totals into the carry chain
                tot = pc3[:, :, CH - 1]
                ykh = sm_p.tile([CH, NDB], dt.float32, tag="ykah")
                # ykh reads PSUM -> must be DVE (GPSIMD has no PSUM access)
                nc.vector.tensor_tensor(ykh[:], tot, comp_old[:],
                                        op=AL.subtract)
                kah.tensor_tensor(c3[:, c + 1, :], c3[:, c, :], ykh[:],
                                  op=AL.add)
                dkh = sm_p.tile([CH, NDB], dt.float32, tag="dkah")
                kah.tensor_tensor(dkh[:], c3[:, c + 1, :], c3[:, c, :],
                                  op=AL.subtract)
                comp_new = sm_p.tile([CH, NDB], dt.float32, tag="ckah")
                kah.tensor_tensor(comp_new[:], dkh[:], ykh[:],
                                  op=AL.subtract)
                comp_old = comp_new

                # spikes = F_t - F_{t-1} in {0,1}, fp16 (per chunk so the
                # DVE never sits on a long op while PE waits for a slot)
                nc.vector.tensor_tensor(
                    s3[:, :, c * CH:(c + 1) * CH],
                    f3[:, :, 1 + c * CH:1 + (c + 1) * CH],
                    f3[:, :, c * CH:(c + 1) * CH], op=AL.subtract)

        bst["carr"] = c3
        bst["comp"] = comp_old
        bst["f"] = f3

        if "out" in skip:
            return
        for c in range(0, CPG, 2):
            _se = getattr(nc, ENGQ[CFG["stq"][(2 * i + c // 2) % len(CFG["stq"])]])
            _se.dma_start(
                y[b, :, t0 + c * CH:t0 + (c + 2) * CH]
                .rearrange("(j p) t -> p j t", p=CH),
                s3[:, :, c * CH:(c + 2) * CH])

    for i in range(NGRP + 1):
        if i < NGRP:
            produce(i)
        if i >= 1:
            consume(i - 1)


def _get_nc():
    if "nc" not in _cache:
        _cache["nc"] = build_nc()
    return _cache["nc"]


def _make_in_maps(x, v0):
    uv = np.triu(np.ones((CH, CH), dtype=np.float32))
    in_maps = []
    for c in range(N_CORES):
        xb = np.ascontiguousarray(x[BC * c:BC * (c + 1)])
        v0b = v0[BC * c:BC * (c + 1)]
        v0tb = np.ascontiguousarray(
            v0b.reshape(BC, NDB, CH).transpose(0, 2, 1).astype(np.float32))
        in_maps.append({"x": xb, "v0t": v0tb, "u": uv})
    return in_maps


def _get_runner():
    """Build (once) a cached jitted SPMD executable over the 8 cores."""
    if "runner" in _cache:
        return _cache["runner"]
    import jax
    from jax.sharding import Mesh, PartitionSpec
    from jax.experimental.shard_map import shard_map
    from concourse import bass2jax

    nc = _get_nc()
    bass2jax.install_neuronx_cc_hook()
    partition_name = nc.partition_id_tensor.name if nc.partition_id_tensor else None
    in_names, out_names, out_avals = [], [], []
    for alloc in nc.m.functions[0].allocations:
        if not isinstance(alloc, mybir.MemoryLocationSet):
            continue
        name = alloc.memorylocations[0].name
        if alloc.kind == "ExternalInput":
            if name != partition_name:
                in_names.append(name)
        elif alloc.kind == "ExternalOutput":
            out_names.append(name)
            out_avals.append(jax.core.ShapedArray(
                tuple(alloc.tensor_shape), dt.np(alloc.dtype)))
    n_params = len(in_names)
    zero_outs = [np.zeros(a.shape, a.dtype) for a in out_avals]
    all_names = in_names + out_names + ([partition_name] if partition_name else [])

    def _body(*args):
        operands = list(args)
        if partition_name is not None:
            operands.append(bass2jax.partition_id_tensor())
        return tuple(bass2jax._bass_exec_p.bind(
            *operands, out_avals=tuple(out_avals), in_names=tuple(all_names),
            out_names=tuple(out_names), lowering_input_output_aliases=(),
            sim_require_finite=True, sim_require_nnan=True, nc=nc))

    devices = jax.devices()[:N_CORES]
    mesh = Mesh(np.asarray(devices), ("core",))
    nin = n_params + len(out_names)
    fn = jax.jit(shard_map(_body, mesh=mesh,
                           in_specs=(PartitionSpec("core"),) * nin,
                           out_specs=(PartitionSpec("core"),) * len(out_names),
                           check_rep=False))
    _cache["runner"] = (fn, in_names, out_names, zero_outs)
    return _cache["runner"]


def kernel(inputs, initial_state):
    import jax
    x = np.ascontiguousarray(np.asarray(inputs, dtype=np.float32))
    v0 = np.ascontiguousarray(np.asarray(initial_state, dtype=np.float32))
    assert x.shape == (B, T, D) and v0.shape == (B, D)
    fn, in_names, out_names, zero_outs = _get_runner()
    in_maps = _make_in_maps(x, v0)
    concat_in = [np.concatenate([np.asarray(in_maps[c][nm])
                                 for c in range(N_CORES)], axis=0)
                 for nm in in_names]
    concat_zero = [np.concatenate([z] * N_CORES, axis=0) for z in zero_outs]
    outs = jax.block_until_ready(fn(*concat_in, *concat_zero))
    ydt = np.asarray(outs[out_names.index("y")])        # [B, D, T] int16 {0,1}
    scale = np.float32(1.0) / np.float32(0.001)         # matches reference /DT
    out = ydt.astype(np.float32).transpose(0, 2, 1) * scale
    return np.ascontiguousarray(out)


# revision 13
# speedup vs baseline: 1.2571x; 1.0387x over previous
"""Trainium2 Bass kernel for DualThresholdSelfregulatingIntegrate.

Computes, matching the jax-on-neuron reference to ~1 moved spike:
    rates  = relu(x) * DT                     # [B, T, D]
    c      = init[:, None, :] + cumsum(rates, axis=1)
    spikes = floor(c) - floor(c_prev)
    out    = spikes / DT

Architecture (per core, pure data-parallel over batch):
  - x loads in natural [t, d] layout; ScalarE computes rates=relu(x)*dt
  - per 128-step chunk, one fp32 PE matmul per d-block computes the
    chunk-local cumsum AND the [t,d]->[d,t'] transpose in one op:
    L[d, t'] = sum_t rates[t, d] * U[t, t'] with U upper-tri ones;
    fp32 PE accumulation reproduces the reference lowering's rounding
  - carries across chunks are Kahan-compensated sums of the chunk totals
    (exact), held per [d_inner(128), j(8)]; the chain starts at v0 - 0.5
    so F = rint(L + carry) = floor(c) directly (int16); per chunk this is
    one DVE STT (carry broadcast along t') and/or per-j ScalarE
    activations (carry as per-partition bias)
  - spike diff along the free (t) axis in {0,1} (int16 -> fp16), written
    per 4-chunk group and DMA'd out in [d, t] layout; the host applies
    the pure layout transpose back to [t, d] and scales {0,1}->{0,1000}
  - output is fp16 (exact for {0,1000}), halving output HBM traffic;
    input DMA alternates between the SP and Pool queues, output DMA
    likewise, so no engine queue carries more than ~38us of DMA
"""

import sys

sys.path.insert(0, "/opt/trn_rl_repo")

import numpy as np

import concourse.bass as bass  # noqa: F401  (registers engines)
import concourse.tile as tile
from concourse import bacc, mybir

N_CORES = 8
B, T, D = 16, 2048, 1024
BC = B // N_CORES          # batches per core
CH = 128                   # chunk (carry granularity, matmul contraction)
G = 512                    # group: time steps per pipeline stage
CPG = G // CH              # 4 chunks per group
NG = T // G                # 4 groups per batch
NDB = D // CH              # 8 d-blocks
dt = mybir.dt

_cache = {}
CFG = {
    "na": 0,        # first `na` d-blocks' F on ScalarE (bias trick), rest DVE
    "relun": 1,     # chunks per relu op
    "kaheng": "v",
    "pcb": 4,       # pc PSUM bufs (2 banks each)
    "ldq": "sg",    # per-group load queue cycle: s=sync, g=gpsimd, a=scalar
    "stq": "gs",    # per-group store queue cycle
}
ENGQ = {"v": "vector", "g": "gpsimd", "s": "sync", "a": "scalar"}


def build_nc(bench_iters=0, skip=()):
    nc = bacc.Bacc("TRN2", target_bir_lowering=False, debug=False)
    x = nc.dram_tensor("x", [BC, T, D], dt.float32, kind="ExternalInput")
    v0t = nc.dram_tensor("v0t", [BC, CH, NDB], dt.float32, kind="ExternalInput")
    u = nc.dram_tensor("u", [CH, CH], dt.float32, kind="ExternalInput")
    y = nc.dram_tensor("y", [BC, D, T], dt.int16, kind="ExternalOutput")

    with tile.TileContext(nc) as tc:
        with tc.tile_pool(name="xin", bufs=3) as xin_p, \
             tc.tile_pool(name="rt", bufs=3) as rt_p, \
             tc.tile_pool(name="ff", bufs=3) as f_p, \
             tc.tile_pool(name="sp", bufs=2) as sp_p, \
             tc.tile_pool(name="cr", bufs=3) as cr_p, \
             tc.tile_pool(name="sm", bufs=5) as sm_p, \
             tc.tile_pool(name="consts", bufs=1) as c_p, \
             tc.tile_pool(name="pc", bufs=CFG["pcb"], space="PSUM") as pc_p:

            ut = c_p.tile([CH, CH], dt.float32, tag="ut")
            nc.sync.dma_start(ut[:], u[:])
            v0tt = c_p.tile([CH, BC * NDB], dt.float32, tag="v0tt")
            nc.sync.dma_start(
                v0tt[:].rearrange("p (b j) -> p b j", b=BC),
                v0t[:].rearrange("b p j -> p b j"),
            )
            v03 = v0tt[:].rearrange("p (b j) -> p b j", b=BC)

            import contextlib
            _hints = (mybir.EngineType.DVE, mybir.EngineType.Activation,
                      mybir.EngineType.PE, mybir.EngineType.SP,
                      mybir.EngineType.Pool)
            loop_cm = tc.For_i(0, bench_iters, 1, hint_engines=_hints) \
                if bench_iters else contextlib.nullcontext()
            with loop_cm:
                body(nc, tc, x, y, v03, ut,
                     xin_p, rt_p, f_p, sp_p, cr_p, sm_p, pc_p,
                     skip=set(skip))
    nc.compile()
    return nc


def body(nc, tc, x, y, v03, ut,
         xin_p, rt_p, f_p, sp_p, cr_p, sm_p, pc_p, skip=()):
    """Software-pipelined emission: for pipeline step i, the produce stage
    (DMA + relu) of group i is emitted BEFORE the consume stage (matmuls,
    floor, carry, diff, store) of group i-1, so ScalarE's relu of the next
    group is never queued behind floor ops that wait on PE."""
    AL = mybir.AluOpType
    AF = mybir.ActivationFunctionType
    kah = getattr(nc, ENGQ[CFG["kaheng"]])
    NA = CFG["na"]
    RN = CFG["relun"]
    NGRP = BC * NG
    state = {}
    bstate = {0: {}, 1: {}}

    def produce(i):
        g, b = divmod(i, BC)
        t0 = g * G
        x4 = xin_p.tile([CH, CPG * D], dt.float32, tag="xk")
        r4 = rt_p.tile([CH, CPG * D], dt.float32, tag="rk")
        for c in range(0, CPG, RN):
            _le = getattr(nc, ENGQ[CFG["ldq"][(2 * i + c // RN) % len(CFG["ldq"])]])
            _le.dma_start(
                x4[:, c * D:(c + RN) * D].rearrange("p (c d) -> p c d", c=RN),
                x[b, t0 + c * CH:t0 + (c + RN) * CH, :]
                .rearrange("(c p) d -> p c d", p=CH))
            nc.scalar.activation(r4[:, c * D:(c + RN) * D],
                                 x4[:, c * D:(c + RN) * D],
                                 AF.Relu, bias=0.0, scale=0.001)
        state[i] = r4

    def consume(i):
        g, b = divmod(i, BC)
        t0 = g * G
        r4 = state.pop(i)
        bst = bstate[b]

        # carry tile: c3[:, c, :] = v0-0.5 + Kahan sum of totals of all
        # chunks before (g, c); col CPG feeds the next group
        carr = cr_p.tile([CH, (CPG + 1) * NDB], dt.float32, tag="carr")
        c3 = carr[:].rearrange("p (c j) -> p c j", c=CPG + 1)
        with tc.high_priority():
            if g == 0:
                nc.vector.tensor_scalar(c3[:, 0, :], v03[:, b, :], -0.5, None,
                                        op0=AL.add)
                comp_old = sm_p.tile([CH, NDB], dt.float32, tag="ckah")
                nc.vector.memset(comp_old[:], 0.0)
            else:
                nc.vector.tensor_copy(c3[:, 0, :], bst["carr"][:, CPG, :])
                comp_old = bst["comp"]

        # F tile for the group (col 0 = prev chunk boundary floor)
        ft = f_p.tile([CH, NDB * (G + 1)], dt.int16, tag="fk")
        f3 = ft[:].rearrange("p (j t) -> p j t", j=NDB)
        with tc.high_priority():
            if g == 0:
                nc.vector.memset(f3[:, :, 0], 0)
            else:
                nc.vector.tensor_copy(f3[:, :, 0], bst["f"][:, :, G])

        spk = sp_p.tile([CH, NDB * G], dt.int16, tag="sk")
        s3 = spk[:].rearrange("p (j t) -> p j t", j=NDB)

        for c in range(CPG):
            # chunk-local cumsum + transpose via one matmul per d-block
            pck = pc_p.tile([CH, D], dt.float32, tag="pck")
            for j in range(NDB):
                nc.tensor.matmul(pck[:, j * CH:(j + 1) * CH],
                                 r4[:, c * D + j * CH:c * D + (j + 1) * CH],
                                 ut[:], start=True, stop=True)
            pc3 = pck[:].rearrange("p (j t) -> p j t", j=NDB)

            with tc.high_priority():
                # F = rint(L + carr) = floor(c), int16
                if "f" not in skip:
                    for j in range(NA):
                        nc.scalar.activation(
                            f3[:, j, 1 + c * CH:1 + (c + 1) * CH],
                            pc3[:, j, :], AF.Relu,
                            bias=c3[:, c, j:j + 1], scale=1.0)
                    if NA < NDB:
                        cb = c3[:, c, NA:].unsqueeze(2).broadcast_to(
                            [CH, NDB - NA, CH])
                        nc.vector.scalar_tensor_tensor(
                            f3[:, NA:, 1 + c * CH:1 + (c + 1) * CH],
                            pc3[:, NA:, :], 1.0, cb,
                            op0=AL.mult, op1=AL.add)

                # Kahan-accumulate chunk totals into the carry chain
                tot = pc3[:, :, CH - 1]
                ykh = sm_p.tile([CH, NDB], dt.float32, tag="ykah")
                # ykh reads PSUM -> must be DVE (GPSIMD has no PSUM access)
                nc.vector.tensor_tensor(ykh[:], tot, comp_old[:],
                                        op=AL.subtract)
                kah.tensor_tensor(c3[:, c + 1, :], c3[:, c, :], ykh[:],
                                  op=AL.add)
                dkh = sm_p.tile([CH, NDB], dt.float32, tag="dkah")
                kah.tensor_tensor(dkh[:], c3[:, c + 1, :], c3[:, c, :],
                                  op=AL.subtract)
                comp_new = sm_p.tile([CH, NDB], dt.float32, tag="ckah")
                kah.tensor_tensor(comp_new[:], dkh[:], ykh[:],
                                  op=AL.subtract)
                comp_old = comp_new

                # spikes = F_t - F_{t-1} in {0,1}, fp16 (per chunk so the
                # DVE never sits on a long op while PE waits for a slot)
                nc.vector.tensor_tensor(
                    s3[:, :, c * CH:(c + 1) * CH],
                    f3[:, :, 1 + c * CH:1 + (c + 1) * CH],
                    f3[:, :, c * CH:(c + 1) * CH], op=AL.subtract)

        bst["carr"] = c3
        bst["comp"] = comp_old
        bst["f"] = f3

        if "out" in skip:
            return
        for c in range(0, CPG, 2):
            _se = getattr(nc, ENGQ[CFG["stq"][(2 * i + c // 2) % len(CFG["stq"])]])
            _se.dma_start(
                y[b, :, t0 + c * CH:t0 + (c + 2) * CH]
                .rearrange("(j p) t -> p j t", p=CH),
                s3[:, :, c * CH:(c + 2) * CH])

    for i in range(NGRP + 1):
        if i < NGRP:
            produce(i)
        if i >= 1:
            consume(i - 1)


def _get_nc():
    if "nc" not in _cache:
        _cache["nc"] = build_nc()
    return _cache["nc"]


def _make_in_maps(x, v0):
    uv = np.triu(np.ones((CH, CH), dtype=np.float32))
    in_maps = []
    for c in range(N_CORES):
        xb = np.ascontiguousarray(x[BC * c:BC * (c + 1)])
        v0b = v0[BC * c:BC * (c + 1)]
        v0tb = np.ascontiguousarray(
            v0b.reshape(BC, NDB, CH).transpose(0, 2, 1).astype(np.float32))
        in_maps.append({"x": xb, "v0t": v0tb, "u": uv})
    return in_maps


def _get_runner():
    """Build (once) a cached jitted SPMD executable over the 8 cores."""
    if "runner" in _cache:
        return _cache["runner"]
    import jax
    from jax.sharding import Mesh, PartitionSpec
    from jax.experimental.shard_map import shard_map
    from concourse import bass2jax

    nc = _get_nc()
    bass2jax.install_neuronx_cc_hook()
    partition_name = nc.partition_id_tensor.name if nc.partition_id_tensor else None
    in_names, out_names, out_avals = [], [], []
    for alloc in nc.m.functions[0].allocations:
        if not isinstance(alloc, mybir.MemoryLocationSet):
            continue
        name = alloc.memorylocations[0].name
        if alloc.kind == "ExternalInput":
            if name != partition_name:
                in_names.append(name)
        elif alloc.kind == "ExternalOutput":
            out_names.append(name)
            out_avals.append(jax.core.ShapedArray(
                tuple(alloc.tensor_shape), dt.np(alloc.dtype)))
    n_params = len(in_names)
    zero_outs = [np.zeros(a.shape, a.dtype) for a in out_avals]
    all_names = in_names + out_names + ([partition_name] if partition_name else [])

    def _body(*args):
        operands = list(args)
        if partition_name is not None:
            operands.append(bass2jax.partition_id_tensor())
        return tuple(bass2jax._bass_exec_p.bind(
            *operands, out_avals=tuple(out_avals), in_names=tuple(all_names),
            out_names=tuple(out_names), lowering_input_output_aliases=(),
            sim_require_finite=True, sim_require_nnan=True, nc=nc))

    devices = jax.devices()[:N_CORES]
    mesh = Mesh(np.asarray(devices), ("core",))
    nin = n_params + len(out_names)
    fn = jax.jit(shard_map(_body, mesh=mesh,
                           in_specs=(PartitionSpec("core"),) * nin,
                           out_specs=(PartitionSpec("core"),) * len(out_names),
                           check_rep=False))
    _cache["runner"] = (fn, in_names, out_names, zero_outs)
    return _cache["runner"]


def kernel(inputs, initial_state):
    import jax
    x = np.ascontiguousarray(np.asarray(inputs, dtype=np.float32))
    v0 = np.ascontiguousarray(np.asarray(initial_state, dtype=np.float32))
    assert x.shape == (B, T, D) and v0.shape == (B, D)
    fn, in_names, out_names, zero_outs = _get_runner()
    in_maps = _make_in_maps(x, v0)
    concat_in = [np.concatenate([np.asarray(in_maps[c][nm])
                                 for c in range(N_CORES)], axis=0)
                 for nm in in_names]
    concat_zero = [np.concatenate([z] * N_CORES, axis=0) for z in zero_outs]
    outs = jax.block_until_ready(fn(*concat_in, *concat_zero))
    ydt = np.asarray(outs[out_names.index("y")])        # [B, D, T] int16 {0,1}
    scale = np.float32(1.0) / np.float32(0.001)         # matches reference /DT
    out = ydt.astype(np.float32).transpose(0, 2, 1) * scale
    return np.ascontiguousarray(out)


# revision 15
# speedup vs baseline: 1.2615x; 1.0035x over previous
"""Trainium2 Bass kernel for DualThresholdSelfregulatingIntegrate.

Computes, matching the jax-on-neuron reference to ~1 moved spike:
    rates  = relu(x) * DT                     # [B, T, D]
    c      = init[:, None, :] + cumsum(rates, axis=1)
    spikes = floor(c) - floor(c_prev)
    out    = spikes / DT

Architecture (per core, pure data-parallel over batch):
  - x loads in natural [t, d] layout; ScalarE computes rates=relu(x)*dt
  - per 128-step chunk, one fp32 PE matmul per d-block computes the
    chunk-local cumsum AND the [t,d]->[d,t'] transpose in one op:
    L[d, t'] = sum_t rates[t, d] * U[t, t'] with U upper-tri ones;
    fp32 PE accumulation reproduces the reference lowering's rounding
  - carries across chunks are Kahan-compensated sums of the chunk totals
    (exact), held per [d_inner(128), j(8)]; the chain starts at v0 - 0.5
    so F = rint(L + carry) = floor(c) directly (int16); per chunk this is
    one DVE STT (carry broadcast along t') and/or per-j ScalarE
    activations (carry as per-partition bias)
  - spike diff along the free (t) axis in {0,1} (int16 -> fp16), written
    per 4-chunk group and DMA'd out in [d, t] layout; the host applies
    the pure layout transpose back to [t, d] and scales {0,1}->{0,1000}
  - output is fp16 (exact for {0,1000}), halving output HBM traffic;
    input DMA alternates between the SP and Pool queues, output DMA
    likewise, so no engine queue carries more than ~38us of DMA
"""

import sys

sys.path.insert(0, "/opt/trn_rl_repo")

import numpy as np

import concourse.bass as bass  # noqa: F401  (registers engines)
import concourse.tile as tile
from concourse import bacc, mybir

N_CORES = 8
B, T, D = 16, 2048, 1024
BC = B // N_CORES          # batches per core
CH = 128                   # chunk (carry granularity, matmul contraction)
G = 512                    # group: time steps per pipeline stage
CPG = G // CH              # 4 chunks per group
NG = T // G                # 4 groups per batch
NDB = D // CH              # 8 d-blocks
dt = mybir.dt

_cache = {}
CFG = {
    "na": 0,        # first `na` d-blocks' F on ScalarE (bias trick), rest DVE
    "relun": 1,     # chunks per relu op
    "kaheng": "v",
    "pcb": 4,       # pc PSUM bufs (2 banks each)
    "ldq": "gs",    # per-group load queue cycle: s=sync, g=gpsimd, a=scalar
    "stq": "sg",    # per-group store queue cycle
}
ENGQ = {"v": "vector", "g": "gpsimd", "s": "sync", "a": "scalar"}


def build_nc(bench_iters=0, skip=()):
    nc = bacc.Bacc("TRN2", target_bir_lowering=False, debug=False)
    x = nc.dram_tensor("x", [BC, T, D], dt.float32, kind="ExternalInput")
    v0t = nc.dram_tensor("v0t", [BC, CH, NDB], dt.float32, kind="ExternalInput")
    u = nc.dram_tensor("u", [CH, CH], dt.float32, kind="ExternalInput")
    y = nc.dram_tensor("y", [BC, D, T], dt.int16, kind="ExternalOutput")

    with tile.TileContext(nc) as tc:
        with tc.tile_pool(name="xin", bufs=3) as xin_p, \
             tc.tile_pool(name="rt", bufs=3) as rt_p, \
             tc.tile_pool(name="ff", bufs=3) as f_p, \
             tc.tile_pool(name="sp", bufs=2) as sp_p, \
             tc.tile_pool(name="cr", bufs=3) as cr_p, \
             tc.tile_pool(name="sm", bufs=5) as sm_p, \
             tc.tile_pool(name="consts", bufs=1) as c_p, \
             tc.tile_pool(name="pc", bufs=CFG["pcb"], space="PSUM") as pc_p:

            ut = c_p.tile([CH, CH], dt.float32, tag="ut")
            nc.sync.dma_start(ut[:], u[:])
            v0tt = c_p.tile([CH, BC * NDB], dt.float32, tag="v0tt")
            nc.sync.dma_start(
                v0tt[:].rearrange("p (b j) -> p b j", b=BC),
                v0t[:].rearrange("b p j -> p b j"),
            )
            v03 = v0tt[:].rearrange("p (b j) -> p b j", b=BC)

            import contextlib
            _hints = (mybir.EngineType.DVE, mybir.EngineType.Activation,
                      mybir.EngineType.PE, mybir.EngineType.SP,
                      mybir.EngineType.Pool)
            loop_cm = tc.For_i(0, bench_iters, 1, hint_engines=_hints) \
                if bench_iters else contextlib.nullcontext()
            with loop_cm:
                body(nc, tc, x, y, v03, ut,
                     xin_p, rt_p, f_p, sp_p, cr_p, sm_p, pc_p,
                     skip=set(skip))
    nc.compile()
    return nc


def body(nc, tc, x, y, v03, ut,
         xin_p, rt_p, f_p, sp_p, cr_p, sm_p, pc_p, skip=()):
    """Software-pipelined emission: for pipeline step i, the produce stage
    (DMA + relu) of group i is emitted BEFORE the consume stage (matmuls,
    floor, carry, diff, store) of group i-1, so ScalarE's relu of the next
    group is never queued behind floor ops that wait on PE."""
    AL = mybir.AluOpType
    AF = mybir.ActivationFunctionType
    kah = getattr(nc, ENGQ[CFG["kaheng"]])
    NA = CFG["na"]
    RN = CFG["relun"]
    NGRP = BC * NG
    state = {}
    bstate = {0: {}, 1: {}}

    def produce(i):
        g, b = divmod(i, BC)
        t0 = g * G
        x4 = xin_p.tile([CH, CPG * D], dt.float32, tag="xk")
        r4 = rt_p.tile([CH, CPG * D], dt.float32, tag="rk")
        for c in range(0, CPG, RN):
            _le = getattr(nc, ENGQ[CFG["ldq"][(2 * i + c // RN) % len(CFG["ldq"])]])
            _le.dma_start(
                x4[:, c * D:(c + RN) * D].rearrange("p (c d) -> p c d", c=RN),
                x[b, t0 + c * CH:t0 + (c + RN) * CH, :]
                .rearrange("(c p) d -> p c d", p=CH))
            nc.scalar.activation(r4[:, c * D:(c + RN) * D],
                                 x4[:, c * D:(c + RN) * D],
                                 AF.Relu, bias=0.0, scale=0.001)
        state[i] = r4

    def consume(i):
        g, b = divmod(i, BC)
        t0 = g * G
        r4 = state.pop(i)
        bst = bstate[b]

        # carry tile: c3[:, c, :] = v0-0.5 + Kahan sum of totals of all
        # chunks before (g, c); col CPG feeds the next group
        carr = cr_p.tile([CH, (CPG + 1) * NDB], dt.float32, tag="carr")
        c3 = carr[:].rearrange("p (c j) -> p c j", c=CPG + 1)
        with tc.high_priority():
            if g == 0:
                nc.vector.tensor_scalar(c3[:, 0, :], v03[:, b, :], -0.5, None,
                                        op0=AL.add)
                comp_old = sm_p.tile([CH, NDB], dt.float32, tag="ckah")
                nc.vector.memset(comp_old[:], 0.0)
            else:
                nc.vector.tensor_copy(c3[:, 0, :], bst["carr"][:, CPG, :])
                comp_old = bst["comp"]

        # F tile for the group (col 0 = prev chunk boundary floor)
        ft = f_p.tile([CH, NDB * (G + 1)], dt.int16, tag="fk")
        f3 = ft[:].rearrange("p (j t) -> p j t", j=NDB)
        with tc.high_priority():
            if g == 0:
                nc.vector.memset(f3[:, :, 0], 0)
            else:
                nc.vector.tensor_copy(f3[:, :, 0], bst["f"][:, :, G])

        spk = sp_p.tile([CH, NDB * G], dt.int16, tag="sk")
        s3 = spk[:].rearrange("p (j t) -> p j t", j=NDB)

        for c in range(CPG):
            # chunk-local cumsum + transpose via one matmul per d-block
            pck = pc_p.tile([CH, D], dt.float32, tag="pck")
            for j in range(NDB):
                nc.tensor.matmul(pck[:, j * CH:(j + 1) * CH],
                                 r4[:, c * D + j * CH:c * D + (j + 1) * CH],
                                 ut[:], start=True, stop=True)
            pc3 = pck[:].rearrange("p (j t) -> p j t", j=NDB)

            with tc.high_priority():
                # F = rint(L + carr) = floor(c), int16
                if "f" not in skip:
                    for j in range(NA):
                        nc.scalar.activation(
                            f3[:, j, 1 + c * CH:1 + (c + 1) * CH],
                            pc3[:, j, :], AF.Relu,
                            bias=c3[:, c, j:j + 1], scale=1.0)
                    if NA < NDB:
                        cb = c3[:, c, NA:].unsqueeze(2).broadcast_to(
                            [CH, NDB - NA, CH])
                        nc.vector.scalar_tensor_tensor(
                            f3[:, NA:, 1 + c * CH:1 + (c + 1) * CH],
                            pc3[:, NA:, :], 1.0, cb,
                            op0=AL.mult, op1=AL.add)

                # Kahan-accumulate chunk totals into the carry chain
                tot = pc3[:, :, CH - 1]
                ykh = sm_p.tile([CH, NDB], dt.float32, tag="ykah")
                # ykh reads PSUM -> must be DVE (GPSIMD has no PSUM access)
                nc.vector.tensor_tensor(ykh[:], tot, comp_old[:],
                                        op=AL.subtract)
                kah.tensor_tensor(c3[:, c + 1, :], c3[:, c, :], ykh[:],
                                  op=AL.add)
                dkh = sm_p.tile([CH, NDB], dt.float32, tag="dkah")
                kah.tensor_tensor(dkh[:], c3[:, c + 1, :], c3[:, c, :],
                                  op=AL.subtract)
                comp_new = sm_p.tile([CH, NDB], dt.float32, tag="ckah")
                kah.tensor_tensor(comp_new[:], dkh[:], ykh[:],
                                  op=AL.subtract)
                comp_old = comp_new

            # spikes = F_t - F_{t-1} in {0,1} (per chunk, normal priority
            # so it yields to the floor/carry recurrence on DVE)
            nc.vector.tensor_tensor(
                s3[:, :, c * CH:(c + 1) * CH],
                f3[:, :, 1 + c * CH:1 + (c + 1) * CH],
                f3[:, :, c * CH:(c + 1) * CH], op=AL.subtract)

        bst["carr"] = c3
        bst["comp"] = comp_old
        bst["f"] = f3

        if "out" in skip:
            return
        for c in range(0, CPG, 2):
            _se = getattr(nc, ENGQ[CFG["stq"][(2 * i + c // 2) % len(CFG["stq"])]])
            _se.dma_start(
                y[b, :, t0 + c * CH:t0 + (c + 2) * CH]
                .rearrange("(j p) t -> p j t", p=CH),
                s3[:, :, c * CH:(c + 2) * CH])

    for i in range(NGRP + 1):
        if i < NGRP:
            produce(i)
        if i >= 1:
            consume(i - 1)


def _get_nc():
    if "nc" not in _cache:
        _cache["nc"] = build_nc()
    return _cache["nc"]


def _make_in_maps(x, v0):
    uv = np.triu(np.ones((CH, CH), dtype=np.float32))
    in_maps = []
    for c in range(N_CORES):
        xb = np.ascontiguousarray(x[BC * c:BC * (c + 1)])
        v0b = v0[BC * c:BC * (c + 1)]
        v0tb = np.ascontiguousarray(
            v0b.reshape(BC, NDB, CH).transpose(0, 2, 1).astype(np.float32))
        in_maps.append({"x": xb, "v0t": v0tb, "u": uv})
    return in_maps


def _get_runner():
    """Build (once) a cached jitted SPMD executable over the 8 cores."""
    if "runner" in _cache:
        return _cache["runner"]
    import jax
    from jax.sharding import Mesh, PartitionSpec
    from jax.experimental.shard_map import shard_map
    from concourse import bass2jax

    nc = _get_nc()
    bass2jax.install_neuronx_cc_hook()
    partition_name = nc.partition_id_tensor.name if nc.partition_id_tensor else None
    in_names, out_names, out_avals = [], [], []
    for alloc in nc.m.functions[0].allocations:
        if not isinstance(alloc, mybir.MemoryLocationSet):
            continue
        name = alloc.memorylocations[0].name
        if alloc.kind == "ExternalInput":
            if name != partition_name:
                in_names.append(name)
        elif alloc.kind == "ExternalOutput":
            out_names.append(name)
            out_avals.append(jax.core.ShapedArray(
                tuple(alloc.tensor_shape), dt.np(alloc.dtype)))
    n_params = len(in_names)
    zero_outs = [np.zeros(a.shape, a.dtype) for a in out_avals]
    all_names = in_names + out_names + ([partition_name] if partition_name else [])

    def _body(*args):
        operands = list(args)
        if partition_name is not None:
            operands.append(bass2jax.partition_id_tensor())
        return tuple(bass2jax._bass_exec_p.bind(
            *operands, out_avals=tuple(out_avals), in_names=tuple(all_names),
            out_names=tuple(out_names), lowering_input_output_aliases=(),
            sim_require_finite=True, sim_require_nnan=True, nc=nc))

    devices = jax.devices()[:N_CORES]
    mesh = Mesh(np.asarray(devices), ("core",))
    nin = n_params + len(out_names)
    fn = jax.jit(shard_map(_body, mesh=mesh,
                           in_specs=(PartitionSpec("core"),) * nin,
                           out_specs=(PartitionSpec("core"),) * len(out_names),
                           check_rep=False))
    _cache["runner"] = (fn, in_names, out_names, zero_outs)
    return _cache["runner"]


def kernel(inputs, initial_state):
    import jax
    x = np.ascontiguousarray(np.asarray(inputs, dtype=np.float32))
    v0 = np.ascontiguousarray(np.asarray(initial_state, dtype=np.float32))
    assert x.shape == (B, T, D) and v0.shape == (B, D)
    fn, in_names, out_names, zero_outs = _get_runner()
    in_maps = _make_in_maps(x, v0)
    concat_in = [np.concatenate([np.asarray(in_maps[c][nm])
                                 for c in range(N_CORES)], axis=0)
                 for nm in in_names]
    concat_zero = [np.concatenate([z] * N_CORES, axis=0) for z in zero_outs]
    outs = jax.block_until_ready(fn(*concat_in, *concat_zero))
    ydt = np.asarray(outs[out_names.index("y")])        # [B, D, T] int16 {0,1}
    scale = np.float32(1.0) / np.float32(0.001)         # matches reference /DT
    out = ydt.astype(np.float32).transpose(0, 2, 1) * scale
    return np.ascontiguousarray(out)


# revision 17
# speedup vs baseline: 1.4746x; 1.1690x over previous
"""Trainium2 Bass kernel for DualThresholdSelfregulatingIntegrate.

Computes, matching the jax-on-neuron reference to ~1 moved spike:
    rates  = relu(x) * DT                     # [B, T, D]
    c      = init[:, None, :] + cumsum(rates, axis=1)
    spikes = floor(c) - floor(c_prev)
    out    = spikes / DT

Architecture (per core, pure data-parallel over batch):
  - x loads in natural [t, d] layout; ScalarE computes rates=relu(x)*dt
  - per 128-step chunk, one fp32 PE matmul per d-block computes the
    chunk-local cumsum AND the [t,d]->[d,t'] transpose in one op:
    L[d, t'] = sum_t rates[t, d] * U[t, t'] with U upper-tri ones;
    fp32 PE accumulation reproduces the reference lowering's rounding
  - carries across chunks are Kahan-compensated sums of the chunk totals
    (exact), held per [d_inner(128), j(8)]; the chain starts at v0 - 0.5
    so F = rint(L + carry) = floor(c) directly (int16); per chunk this is
    one DVE STT (carry broadcast along t') and/or per-j ScalarE
    activations (carry as per-partition bias)
  - spike diff along the free (t) axis in {0,1} int16, written per chunk
    and DMA'd out in [d, t] layout; the host applies the pure layout
    transpose back to [t, d] and the exact f32(1)/f32(0.001) scale
  - int16 output halves output HBM traffic (25.2 vs 33.6 MB per core);
    input/output DMA alternates between the SP and Pool queues so no
    queue carries more than ~38us of DMA
  - the two batches per core are interleaved group-by-group so their
    independent Kahan carry recurrences overlap; floor/carry ops run at
    high Tile-scheduler priority so PSUM slots recycle fast enough to
    keep PE streaming at its fast p-state
"""

import sys

sys.path.insert(0, "/opt/trn_rl_repo")

import numpy as np

import concourse.bass as bass  # noqa: F401  (registers engines)
import concourse.tile as tile
from concourse import bacc, mybir

N_CORES = 8
B, T, D = 16, 2048, 1024
BC = B // N_CORES          # batches per core
CH = 128                   # chunk (carry granularity, matmul contraction)
G = 512                    # group: time steps per pipeline stage
CPG = G // CH              # 4 chunks per group
NG = T // G                # 4 groups per batch
NDB = D // CH              # 8 d-blocks
dt = mybir.dt

_cache = {}
CFG = {
    "na": 0,        # first `na` d-blocks' F on ScalarE (bias trick), rest DVE
    "relun": 1,     # chunks per relu op
    "kaheng": "v",
    "pcb": 2,       # pc PSUM bufs (4 banks each, joint-batch)
    "ldq": "gs",    # per-group load queue cycle: s=sync, g=gpsimd, a=scalar
    "stq": "sg",    # per-group store queue cycle
}
ENGQ = {"v": "vector", "g": "gpsimd", "s": "sync", "a": "scalar"}


def build_nc(bench_iters=0, skip=()):
    nc = bacc.Bacc("TRN2", target_bir_lowering=False, debug=False)
    x = nc.dram_tensor("x", [BC, T, D], dt.float32, kind="ExternalInput")
    v0t = nc.dram_tensor("v0t", [BC, CH, NDB], dt.float32, kind="ExternalInput")
    u = nc.dram_tensor("u", [CH, CH], dt.float32, kind="ExternalInput")
    y = nc.dram_tensor("y", [BC, D, T], dt.int16, kind="ExternalOutput")

    with tile.TileContext(nc) as tc:
        with tc.tile_pool(name="xin", bufs=4) as xin_p, \
             tc.tile_pool(name="rt", bufs=4) as rt_p, \
             tc.tile_pool(name="ff", bufs=2) as f_p, \
             tc.tile_pool(name="sp", bufs=2) as sp_p, \
             tc.tile_pool(name="cr", bufs=2) as cr_p, \
             tc.tile_pool(name="sm", bufs=3) as sm_p, \
             tc.tile_pool(name="consts", bufs=1) as c_p, \
             tc.tile_pool(name="pc", bufs=CFG["pcb"], space="PSUM") as pc_p:

            ut = c_p.tile([CH, CH], dt.float32, tag="ut")
            nc.sync.dma_start(ut[:], u[:])
            v0tt = c_p.tile([CH, BC * NDB], dt.float32, tag="v0tt")
            nc.sync.dma_start(
                v0tt[:].rearrange("p (b j) -> p b j", b=BC),
                v0t[:].rearrange("b p j -> p b j"),
            )
            v03 = v0tt[:].rearrange("p (b j) -> p b j", b=BC)

            import contextlib
            _hints = (mybir.EngineType.DVE, mybir.EngineType.Activation,
                      mybir.EngineType.PE, mybir.EngineType.SP,
                      mybir.EngineType.Pool)
            loop_cm = tc.For_i(0, bench_iters, 1, hint_engines=_hints) \
                if bench_iters else contextlib.nullcontext()
            with loop_cm:
                body(nc, tc, x, y, v03, ut,
                     xin_p, rt_p, f_p, sp_p, cr_p, sm_p, pc_p,
                     skip=set(skip))
    nc.compile()
    return nc


def body(nc, tc, x, y, v03, ut,
         xin_p, rt_p, f_p, sp_p, cr_p, sm_p, pc_p, skip=()):
    """Both batches' chunk c share one joint PSUM tile and joint floor/
    carry/diff ops, halving the serial carry-recurrence op count and
    semaphore hops; produce (DMA + relu) stays per (group, batch) and
    runs one pipeline step ahead."""
    AL = mybir.AluOpType
    AF = mybir.ActivationFunctionType
    RN = CFG["relun"]
    state = {}
    bstate = {}

    def produce(g, b):
        t0 = g * G
        x4 = xin_p.tile([CH, CPG * D], dt.float32, tag="xk")
        r4 = rt_p.tile([CH, CPG * D], dt.float32, tag="rk")
        for c in range(0, CPG, RN):
            _le = getattr(nc, ENGQ[CFG["ldq"][(2 * g + b + c // RN) % len(CFG["ldq"])]])
            _le.dma_start(
                x4[:, c * D:(c + RN) * D].rearrange("p (c d) -> p c d", c=RN),
                x[b, t0 + c * CH:t0 + (c + RN) * CH, :]
                .rearrange("(c p) d -> p c d", p=CH))
            nc.scalar.activation(r4[:, c * D:(c + RN) * D],
                                 x4[:, c * D:(c + RN) * D],
                                 AF.Relu, bias=0.0, scale=0.001)
        state[(g, b)] = r4

    def consume(g):
        t0 = g * G
        r4s = [state.pop((g, b)) for b in range(BC)]

        # joint carry tile: c3[:, c, :] = [b, j]-carry before chunk (g, c)
        carr = cr_p.tile([CH, (CPG + 1) * BC * NDB], dt.float32, tag="carr")
        c3 = carr[:].rearrange("p (c r) -> p c r", c=CPG + 1)
        with tc.high_priority():
            if g == 0:
                nc.vector.tensor_scalar(
                    c3[:, 0, :],
                    v03[:, :, :].rearrange("p b j -> p (b j)"),
                    -0.5, None, op0=AL.add)
                comp_old = sm_p.tile([CH, BC * NDB], dt.float32, tag="ckah")
                nc.vector.memset(comp_old[:], 0.0)
            else:
                nc.vector.tensor_copy(c3[:, 0, :], bstate["carr"][:, CPG, :])
                comp_old = bstate["comp"]

        # joint F tile: [p, (b, j, t)] with col 0 = prev chunk boundary
        ft = f_p.tile([CH, BC * NDB * (G + 1)], dt.int16, tag="fk")
        f4 = ft[:].rearrange("p (b j t) -> p b j t", b=BC, j=NDB)
        f3 = ft[:].rearrange("p (r t) -> p r t", t=G + 1)
        with tc.high_priority():
            if g == 0:
                nc.vector.memset(f3[:, :, 0], 0)
            else:
                nc.vector.tensor_copy(f3[:, :, 0], bstate["f"][:, :, G])

        spk = sp_p.tile([CH, BC * NDB * G], dt.int16, tag="sk")
        s4 = spk[:].rearrange("p (b j t) -> p b j t", b=BC, j=NDB)
        s3 = spk[:].rearrange("p (r t) -> p r t", t=G)

        for c in range(CPG):
            # joint chunk tile: both batches side by side (4 PSUM banks)
            pck = pc_p.tile([CH, BC * D], dt.float32, tag="pck")
            for b in range(BC):
                for j in range(NDB):
                    nc.tensor.matmul(
                        pck[:, b * D + j * CH:b * D + (j + 1) * CH],
                        r4s[b][:, c * D + j * CH:c * D + (j + 1) * CH],
                        ut[:], start=True, stop=True)
            pc3 = pck[:].rearrange("p (r t) -> p r t", t=CH)

            with tc.high_priority():
                # F = rint(L + carr) = floor(c), int16, both batches
                if "f" not in skip:
                    cb = c3[:, c, :].unsqueeze(2).broadcast_to(
                        [CH, BC * NDB, CH])
                    nc.vector.scalar_tensor_tensor(
                        f3[:, :, 1 + c * CH:1 + (c + 1) * CH],
                        pc3, 1.0, cb, op0=AL.mult, op1=AL.add)

                # joint Kahan carry update ([128, 16], one chain)
                tot = pc3[:, :, CH - 1]
                ykh = sm_p.tile([CH, BC * NDB], dt.float32, tag="ykah")
                nc.vector.tensor_tensor(ykh[:], tot, comp_old[:],
                                        op=AL.subtract)
                nc.vector.tensor_tensor(c3[:, c + 1, :], c3[:, c, :], ykh[:],
                                        op=AL.add)
                dkh = sm_p.tile([CH, BC * NDB], dt.float32, tag="dkah")
                nc.vector.tensor_tensor(dkh[:], c3[:, c + 1, :], c3[:, c, :],
                                        op=AL.subtract)
                comp_new = sm_p.tile([CH, BC * NDB], dt.float32, tag="ckah")
                nc.vector.tensor_tensor(comp_new[:], dkh[:], ykh[:],
                                        op=AL.subtract)
                comp_old = comp_new

            # spikes = F_t - F_{t-1} in {0,1} (joint, normal priority)
            nc.vector.tensor_tensor(
                s3[:, :, c * CH:(c + 1) * CH],
                f3[:, :, 1 + c * CH:1 + (c + 1) * CH],
                f3[:, :, c * CH:(c + 1) * CH], op=AL.subtract)

        bstate["carr"] = c3
        bstate["comp"] = comp_old
        bstate["f"] = f3

        if "out" in skip:
            return
        for b in range(BC):
            for c in range(0, CPG, 2):
                _se = getattr(nc, ENGQ[CFG["stq"][(2 * g + b + c // 2) % len(CFG["stq"])]])
                _se.dma_start(
                    y[b, :, t0 + c * CH:t0 + (c + 2) * CH]
                    .rearrange("(j p) t -> p j t", p=CH),
                    s4[:, b, :, c * CH:(c + 2) * CH])

    for s in range(NG + 1):
        if s < NG:
            for b in range(BC):
                produce(s, b)
        if s >= 1:
            consume(s - 1)


def _get_nc():
    if "nc" not in _cache:
        _cache["nc"] = build_nc()
    return _cache["nc"]


def _make_in_maps(x, v0):
    uv = np.triu(np.ones((CH, CH), dtype=np.float32))
    in_maps = []
    for c in range(N_CORES):
        xb = np.ascontiguousarray(x[BC * c:BC * (c + 1)])
        v0b = v0[BC * c:BC * (c + 1)]
        v0tb = np.ascontiguousarray(
            v0b.reshape(BC, NDB, CH).transpose(0, 2, 1).astype(np.float32))
        in_maps.append({"x": xb, "v0t": v0tb, "u": uv})
    return in_maps


def _get_runner():
    """Build (once) a cached jitted SPMD executable over the 8 cores."""
    if "runner" in _cache:
        return _cache["runner"]
    import jax
    from jax.sharding import Mesh, PartitionSpec
    from jax.experimental.shard_map import shard_map
    from concourse import bass2jax

    nc = _get_nc()
    bass2jax.install_neuronx_cc_hook()
    partition_name = nc.partition_id_tensor.name if nc.partition_id_tensor else None
    in_names, out_names, out_avals = [], [], []
    for alloc in nc.m.functions[0].allocations:
        if not isinstance(alloc, mybir.MemoryLocationSet):
            continue
        name = alloc.memorylocations[0].name
        if alloc.kind == "ExternalInput":
            if name != partition_name:
                in_names.append(name)
        elif alloc.kind == "ExternalOutput":
            out_names.append(name)
            out_avals.append(jax.core.ShapedArray(
                tuple(alloc.tensor_shape), dt.np(alloc.dtype)))
    n_params = len(in_names)
    zero_outs = [np.zeros(a.shape, a.dtype) for a in out_avals]
    all_names = in_names + out_names + ([partition_name] if partition_name else [])

    def _body(*args):
        operands = list(args)
        if partition_name is not None:
            operands.append(bass2jax.partition_id_tensor())
        return tuple(bass2jax._bass_exec_p.bind(
            *operands, out_avals=tuple(out_avals), in_names=tuple(all_names),
            out_names=tuple(out_names), lowering_input_output_aliases=(),
            sim_require_finite=True, sim_require_nnan=True, nc=nc))

    devices = jax.devices()[:N_CORES]
    mesh = Mesh(np.asarray(devices), ("core",))
    nin = n_params + len(out_names)
    fn = jax.jit(shard_map(_body, mesh=mesh,
                           in_specs=(PartitionSpec("core"),) * nin,
                           out_specs=(PartitionSpec("core"),) * len(out_names),
                           check_rep=False))
    _cache["runner"] = (fn, in_names, out_names, zero_outs)
    return _cache["runner"]


def kernel(inputs, initial_state):
    import jax
    x = np.ascontiguousarray(np.asarray(inputs, dtype=np.float32))
    v0 = np.ascontiguousarray(np.asarray(initial_state, dtype=np.float32))
    assert x.shape == (B, T, D) and v0.shape == (B, D)
    fn, in_names, out_names, zero_outs = _get_runner()
    in_maps = _make_in_maps(x, v0)
    concat_in = [np.concatenate([np.asarray(in_maps[c][nm])
                                 for c in range(N_CORES)], axis=0)
                 for nm in in_names]
    concat_zero = [np.concatenate([z] * N_CORES, axis=0) for z in zero_outs]
    outs = jax.block_until_ready(fn(*concat_in, *concat_zero))
    ydt = np.asarray(outs[out_names.index("y")])        # [B, D, T] int16 {0,1}
    scale = np.float32(1.0) / np.float32(0.001)         # matches reference /DT
    out = ydt.astype(np.float32).transpose(0, 2, 1) * scale
    return np.ascontiguousarray(out)
